# revision 23
# baseline (speedup 1.0000x reference)
"""Trainium2 Bass kernel for the 2-layer ChebConv (K=4) GNN with graph pooling.

Strategy (8 NeuronCores, SPMD single program):
  - Nodes sharded into 8 contiguous slabs by destination; each core owns the
    segmented-sum rows for its slab.  Edge structure preprocessed on host into
    (chunk x 1024-dest-block x fixed-128-dest-window) tiles of 128 edges.
  - Sparse L_hat @ h: u kept FEATURE-MAJOR everywhere.  Per source chunk the
    scaled u slab is loaded into a [F, 12500] fp32 SBUF table; gpsimd
    ap_gather streams per-edge feature columns out of it.  Messages are
    converted to fp16 (scalar engine), XBAR DMA-transposed to edge-major
    [128, T, F] tiles, then scatter-accumulated into PSUM dest windows by PE
    matmuls against HOST-PREBUILT one-hot tiles (values = raw edge weights,
    placed by edge structure - pure layout, no host float math).
  - diag of L_hat is exactly 0 (2/lambda_max - 1), so L_hat@h is pure scatter.
  - w_e = -dis[row]*ew*dis[col]: dis[row] folded into the AllGathered u,
    ew into the one-hot values, -dis[col] (and the Chebyshev 2x) applied on
    the PSUM->accumulator path via a broadcast-row multiply.
  - deg = segment_sum(ew, row): prebuilt one-hot scatter matmuls, ones
    stationary, fixed 64-wide windows.
  - u_k = dis*Tx_k AllGathered feature-major between rounds.
  - Pooling: h2 AllGathered feature-major; per-graph segment reduces with
    compile-time global graph boundaries; linear head on device.
"""

import math
import numpy as np

NC = 8
BLKS = 1024        # scatter dest-block (PSUM accumulator width)
BLKC = 512         # conv/scale block width
TILE = 128
WSC = 128          # scatter window width
WDG = 64           # degree window width
G_FIXED = 256
CAP = 16           # max tiles per gather unit (SBUF budget)


# ----------------------------------------------------------------------------
# Host-side preprocessing (integer / layout only -- no float arithmetic)
# ----------------------------------------------------------------------------

def _prep_scatter(row, col, ew, N, S, chunk):
    """Per-core structures for the scatter segmented sums.

    Tiles of <=128 edges; each tile targets one fixed WSC-wide dest window of
    one dest block, sources within one chunk.  Tile counts padded to the max
    across cores per (chunk, block, window) so the SPMD program is uniform.
    One-hot tiles [128, WSC] are prebuilt on host (values = raw edge weight).
    """
    nblk = math.ceil(S / BLKS)
    nch = N // chunk
    nwin = BLKS // WSC
    percore = []
    for i in range(NC):
        sel = np.nonzero((col // S) == i)[0]
        dloc = (col[sel] - i * S).astype(np.int64)
        ch = row[sel] // chunk
        b = dloc // BLKS
        w = (dloc % BLKS) // WSC
        key = ((ch * nblk + b) * nwin + w)
        order = np.argsort(key * (2 * N) + row[sel], kind="stable")
        sel = sel[order]
        cnt = np.bincount(key[order], minlength=nch * nblk * nwin)
        bound = np.concatenate([[0], np.cumsum(cnt)])
        percore.append((sel, bound))

    # padded tile counts per (ch, b, w)
    T = np.zeros((nch, nblk, nwin), np.int64)
    for i in range(NC):
        cnt = np.diff(percore[i][1]).reshape(nch, nblk, nwin)
        T = np.maximum(T, -(-cnt // TILE))
    blkw = [min(BLKS, S - b * BLKS) for b in range(nblk)]

    # cell = (ch, b); units split cells into even gather groups of <=CAP tiles
    cells = []
    t0 = 0
    for ch in range(nch):
        for b in range(nblk):
            tc = int(T[ch, b].sum())
            if tc == 0:
                continue
            wins = []
            for w in range(nwin):
                wins += [w] * int(T[ch, b, w])
            nu = -(-tc // CAP)
            base = tc // nu
            rem = tc - base * nu
            units = []
            u0 = 0
            for k in range(nu):
                un = base + (1 if k < rem else 0)
                units.append((u0, un))
                u0 += un
            cells.append(dict(ch=ch, b=b, t0=t0, tc=tc, wins=wins, units=units))
            t0 += tc
    TOT = t0

    idxcols = sum(u[1] * (TILE // 16) for c in cells for u in c["units"])

    out = []
    for i in range(NC):
        sel, bound = percore[i]
        ohs = np.zeros((TILE, TOT * WSC), np.float16)
        srel = np.zeros((TOT, TILE), np.int64)
        for c in cells:
            ch, b = c["ch"], c["b"]
            t = c["t0"]
            for w in range(nwin):
                k = (ch * nblk + b) * nwin + w
                lo, hi = int(bound[k]), int(bound[k + 1])
                eids = sel[lo:hi]
                for tt in range(int(T[ch, b, w])):
                    e = eids[tt * TILE : (tt + 1) * TILE]
                    kk = len(e)
                    if kk:
                        p = np.arange(kk)
                        dr = (col[e] - i * S - b * BLKS - w * WSC)
                        ohs[p, t * WSC + dr] = ew[e].astype(np.float16)
                        srel[t, :kk] = row[e] - ch * chunk
                    t += 1

        # gather index stream: per unit, wrapped [16, n/16], cores 0-3
        idxw = np.zeros((64, idxcols), np.int16)
        c0 = 0
        for c in cells:
            for (u0, un) in c["units"]:
                cols = un * (TILE // 16)
                flat = srel[c["t0"] + u0 : c["t0"] + u0 + un].reshape(-1)
                wrap = np.zeros((16, cols), np.int16)
                ssl = np.arange(un * TILE)
                wrap[ssl % 16, ssl // 16] = flat.astype(np.int16)
                for r0 in range(0, 64, 16):
                    idxw[r0 : r0 + 16, c0 : c0 + cols] = wrap
                c0 += cols
        out.append(dict(ohs=ohs, idxw=idxw))
    return out, cells, TOT, idxcols, blkw


def _prep_deg(row, ew, N, S):
    """Per-core row-sorted prebuilt one-hots for the degree computation
    (fixed WDG-wide windows on the 1024 grid)."""
    nblk = math.ceil(S / BLKS)
    nwin = BLKS // WDG
    percore = []
    for i in range(NC):
        sel = np.nonzero((row // S) == i)[0]
        rloc = (row[sel] - i * S).astype(np.int64)
        key = (rloc // BLKS) * nwin + (rloc % BLKS) // WDG
        order = np.argsort(key, kind="stable")
        sel = sel[order]
        cnt = np.bincount(key[order], minlength=nblk * nwin)
        bound = np.concatenate([[0], np.cumsum(cnt)])
        percore.append((sel, bound))
    T2 = np.zeros((nblk, nwin), np.int64)
    for i in range(NC):
        cnt = np.diff(percore[i][1]).reshape(nblk, nwin)
        T2 = np.maximum(T2, -(-cnt // TILE))
    TOT2 = int(T2.sum())
    out = []
    for i in range(NC):
        sel, bound = percore[i]
        ohd = np.zeros((TILE, TOT2 * WDG), np.float16)
        t = 0
        for b in range(nblk):
            for w in range(nwin):
                k = b * nwin + w
                lo, hi = int(bound[k]), int(bound[k + 1])
                eids = sel[lo:hi]
                for tt in range(int(T2[b, w])):
                    e = eids[tt * TILE : (tt + 1) * TILE]
                    kk = len(e)
                    if kk:
                        p = np.arange(kk)
                        dr = (row[e] - i * S - b * BLKS - w * WDG)
                        ohd[p, t * WDG + dr] = ew[e].astype(np.float16)
                    t += 1
        out.append(dict(ohd=ohd))
    return out, T2, TOT2


# ----------------------------------------------------------------------------
# Bass program
# ----------------------------------------------------------------------------

def _build(cfg):
    from concourse import bass, bacc, tile, mybir, library_config
    import contextlib

    f32 = mybir.dt.float32
    f16 = mybir.dt.float16
    i16 = mybir.dt.int16

    N, S, F, G = cfg["N"], cfg["S"], cfg["F"], cfg["G"]
    K = cfg["K"]
    CHUNK = cfg["CHUNK"]
    NCH = N // CHUNK
    cells = cfg["cells"]
    TOT, IDXCOLS = cfg["TOT"], cfg["IDXCOLS"]
    T2, TOT2 = cfg["T2"], cfg["TOT2"]
    blkw = cfg["blkw"]
    nblk = len(blkw)
    nwin2 = BLKS // WDG
    nblkc = math.ceil(S / BLKC)
    blkwc = [min(BLKC, S - b * BLKC) for b in range(nblkc)]
    gb = cfg["graph_bounds"]

    nc = bacc.Bacc("TRN2", target_bir_lowering=False, debug=False,
                   num_devices=NC)

    def din(name, shape, dt):
        return nc.dram_tensor(name, shape, dt, kind="ExternalInput")

    t_xfm = din("x_fm", [F, S], f32)
    t_xfm16 = din("x_fm16", [F, S], f16)
    t_idxw = din("idxw", [64, IDXCOLS], i16)
    t_ohs = din("ohs", [TILE, TOT * WSC], f16)
    t_ohd = din("ohd", [TILE, TOT2 * WDG], f16)
    t_w1 = din("w1", [F, K * F], f16)
    t_w2 = din("w2", [F, K * F], f16)
    t_b1 = din("b1c", [F, 1], f32)
    t_b2 = din("b2c", [F, 1], f32)
    t_gam = din("gam", [F, 1], f32)
    t_bet = din("bet", [F, 1], f32)
    t_mu = din("muv", [F, 1], f32)
    t_var = din("varv", [F, 1], f32)
    t_linw = din("linwt", [F, 6], f32)
    t_linb = din("linbc", [2, 1], f32)
    t_cnt = din("cntf", [1, G], f32)
    t_out = nc.dram_tensor("out", [2, G], f32, kind="ExternalOutput")

    rg = [list(range(NC))]

    with tile.TileContext(nc) as tc:
        ctx = contextlib.ExitStack()
        with ctx:
            sb = ctx.enter_context(tc.tile_pool(name="sb", bufs=1))
            ps = ctx.enter_context(tc.tile_pool(name="ps", bufs=1, space="PSUM"))
            dr = ctx.enter_context(tc.tile_pool(name="dr", bufs=1, space="DRAM"))

            nc.gpsimd.load_library(library_config.ap_gather)

            # Dummy-gather machinery: the Pool/Q7 engine has a very expensive
            # (~45us) wake-up whenever it blocks on a semaphore.  Cheap junk
            # gathers between real ones keep it awake and pace its queue so
            # real gathers arrive with their inputs already available.
            import os
            ND = int(os.environ.get("ND", "4"))
            dmy_tab = sb.tile([F, 256], f32, name="dmy_tab")
            nc.vector.memset(dmy_tab[:], 0.0)
            dmy_idx = sb.tile([F, 128], i16, name="dmy_idx")
            nc.vector.memset(dmy_idx[:], 0)
            dmy_out = sb.tile([F, 1024], f32, name="dmy_out")

            def dummy_gathers(n=ND):
                for _ in range(n):
                    nc.gpsimd.ap_gather(
                        out_ap=dmy_out[:, :1024],
                        in_ap=dmy_tab[:, :],
                        idxs_ap=dmy_idx[:, :64],
                        channels=F, num_elems=256, d=1, num_idxs=1024)

            # ---------------- persistent loads ----------------
            ones16 = sb.tile([TILE, 1], f16)
            nc.vector.memset(ones16[:], 1.0)
            ones1f = sb.tile([1, F], f32)
            nc.vector.memset(ones1f[:], 1.0)
            w1_sb = sb.tile([F, K * F], f16)
            nc.sync.dma_start(out=w1_sb[:], in_=t_w1[:, :])
            w2_sb = sb.tile([F, K * F], f16)
            nc.sync.dma_start(out=w2_sb[:], in_=t_w2[:, :])
            b1_sb = sb.tile([F, 1], f32)
            nc.sync.dma_start(out=b1_sb[:], in_=t_b1[:, :])
            b2_sb = sb.tile([F, 1], f32)
            nc.sync.dma_start(out=b2_sb[:], in_=t_b2[:, :])
            linw_sb = sb.tile([F, 6], f32)
            nc.sync.dma_start(out=linw_sb[:], in_=t_linw[:, :])
            linb_sb = sb.tile([2, 1], f32)
            nc.sync.dma_start(out=linb_sb[:], in_=t_linb[:, :])
            cnt_sb = sb.tile([1, G], f32)
            nc.sync.dma_start(out=cnt_sb[:], in_=t_cnt[:, :])

            gam_sb = sb.tile([F, 1], f32)
            nc.sync.dma_start(out=gam_sb[:], in_=t_gam[:, :])
            bet_sb = sb.tile([F, 1], f32)
            nc.sync.dma_start(out=bet_sb[:], in_=t_bet[:, :])
            mu_sb = sb.tile([F, 1], f32)
            nc.sync.dma_start(out=mu_sb[:], in_=t_mu[:, :])
            var_sb = sb.tile([F, 1], f32)
            nc.sync.dma_start(out=var_sb[:], in_=t_var[:, :])
            bnscale = sb.tile([F, 1], f32)
            bnbias = sb.tile([F, 1], f32)
            tmp1 = sb.tile([F, 1], f32)
            nc.vector.tensor_scalar_add(tmp1[:], var_sb[:], 1e-5)
            nc.vector.reciprocal(tmp1[:], tmp1[:])
            nc.scalar.sqrt(tmp1[:], tmp1[:])
            nc.vector.tensor_mul(bnscale[:], gam_sb[:], tmp1[:])
            nc.vector.tensor_mul(tmp1[:], bnscale[:], mu_sb[:])
            nc.vector.tensor_sub(bnbias[:], bet_sb[:], tmp1[:])

            dis_dram = dr.tile([1, S], f32)
            dism1_dram = dr.tile([1, S], f32)
            dism2_dram = dr.tile([1, S], f32)
            h1_dram = dr.tile([F, S], f16)
            tx1_dram = dr.tile([F, S], f16)
            oacc_dram = dr.tile([F, S], f32)
            u_full = [dr.tile([NC * F, S], f32, addr_space="Shared",
                              name=f"u_full{r}") for r in range(6)]
            h2_full = dr.tile([NC * F, S], f32, addr_space="Shared",
                              name="h2_full")

            # big persistent SBUF tensors
            y_acc = sb.tile([F, S], f16, name="y_acc")
            table = sb.tile([F, CHUNK], f32, name="utable")

            # ---------------- deg phase ----------------
            t0 = 0
            for b in range(nblk):
                bw = blkw[b]
                dps = ps.tile([F, BLKS], f32, tag="sps", bufs=2)
                nc.vector.memset(dps[0:1, :], 0.0)
                Tball = int(T2[b].sum())
                done = 0
                for w in range(nwin2):
                    Tw = int(T2[b][w])
                    for gs in range(0, Tw, 16):
                        gn = min(16, Tw - gs)
                        tg = t0 + done + gs
                        doh = sb.tile([TILE, 16 * WDG], f16, tag="doh", bufs=2)
                        nc.sync.dma_start(
                            out=doh[:, : gn * WDG],
                            in_=t_ohd[:, tg * WDG : (tg + gn) * WDG])
                        for j in range(gn):
                            t = done + gs + j
                            nc.tensor.matmul(
                                out=dps[0:1, w * WDG : (w + 1) * WDG],
                                lhsT=ones16[:],
                                rhs=doh[:, j * WDG : (j + 1) * WDG],
                                start=False, stop=(t == Tball - 1),
                                skip_group_check=True)
                    done += Tw
                drow = sb.tile([1, BLKS], f32, tag="drow", bufs=2)
                mrow = sb.tile([1, BLKS], f32, tag="mrow", bufs=2)
                nc.vector.tensor_scalar(
                    out=mrow[0:1, :bw], in0=dps[0:1, :bw], scalar1=0.0,
                    scalar2=None, op0=mybir.AluOpType.is_gt)
                nc.vector.tensor_scalar_max(drow[0:1, :bw], dps[0:1, :bw], 1e-30)
                nc.vector.reciprocal(drow[0:1, :bw], drow[0:1, :bw])
                nc.scalar.sqrt(drow[0:1, :bw], drow[0:1, :bw])
                nc.vector.tensor_mul(drow[0:1, :bw], drow[0:1, :bw], mrow[0:1, :bw])
                nc.sync.dma_start(out=dis_dram[0:1, b * BLKS : b * BLKS + bw],
                                  in_=drow[0:1, :bw])
                nc.vector.tensor_scalar_mul(mrow[0:1, :bw], drow[0:1, :bw], -1.0)
                nc.sync.dma_start(out=dism1_dram[0:1, b * BLKS : b * BLKS + bw],
                                  in_=mrow[0:1, :bw])
                nc.vector.tensor_scalar_mul(mrow[0:1, :bw], drow[0:1, :bw], -2.0)
                nc.sync.dma_start(out=dism2_dram[0:1, b * BLKS : b * BLKS + bw],
                                  in_=mrow[0:1, :bw])
                t0 += Tball

            # ---------------- round helpers ----------------
            def rep_row(src_dram, b, bw):
                """PSUM [F, bw] broadcast of a DRAM row slice (conv grid)."""
                dm_t = sb.tile([1, BLKC], f32, tag="dm_t", bufs=2)
                nc.sync.dma_start(out=dm_t[0:1, :bw],
                                  in_=src_dram[0:1, b * BLKC : b * BLKC + bw])
                rep = ps.tile([F, BLKC], f32, tag="rep", bufs=1)
                nc.tensor.matmul(out=rep[:F, :bw], lhsT=ones1f[:],
                                 rhs=dm_t[0:1, :bw], start=True, stop=True)
                return rep

            def scale_to_u_and_ag(use_yacc, rnd):
                """u = dis * tx, feature-major, AllGather to u_full[rnd]."""
                ag_in = dr.tile([F, S], f32, tag="ag_in", bufs=2,
                                name=f"agin{rnd}")
                for b in range(nblkc):
                    bw = blkwc[b]
                    rep = rep_row(dis_dram, b, bw)
                    if use_yacc:
                        srcap = y_acc[:, b * BLKC : b * BLKC + bw]
                    else:
                        st = sb.tile([F, BLKC], f32, tag="ust", bufs=2)
                        nc.sync.dma_start(out=st[:, :bw],
                                          in_=t_xfm[:, b * BLKC : b * BLKC + bw])
                        srcap = st[:, :bw]
                    stg = sb.tile([F, BLKC], f32, tag="stg", bufs=2)
                    nc.vector.tensor_tensor(out=stg[:, :bw], in0=srcap,
                                            in1=rep[:F, :bw],
                                            op=mybir.AluOpType.mult)
                    nc.sync.dma_start(out=ag_in[:, b * BLKC : b * BLKC + bw],
                                      in_=stg[:, :bw])
                nc.gpsimd.collective_compute(
                    "AllGather", mybir.AluOpType.bypass, replica_groups=rg,
                    ins=[ag_in[:]], outs=[u_full[rnd][:, :]])

            def seg_sum_round(rnd, dism_tag):
                """y_acc = (L_hat @ u) scaled; fp16 [F, S]."""
                nc.vector.memset(y_acc[:], 0.0)
                ic0 = [0]
                for ch in range(NCH):
                    nc.sync.dma_start(out=table[:, :],
                                      in_=u_full[rnd][ch * F : (ch + 1) * F, :])
                    for c in cells:
                        if c["ch"] != ch:
                            continue
                        b = c["b"]
                        bw = blkw[b]
                        sps = ps.tile([F, BLKS], f32, tag="sps", bufs=2)
                        nc.vector.memset(sps[:], 0.0)
                        tc_ = c["tc"]
                        wins = c["wins"]
                        mm = 0
                        for (u0, un) in c["units"]:
                            nidx = un * TILE
                            cols = un * (TILE // 16)
                            idxt = sb.tile([64, CAP * 8], i16,
                                           tag="idxt", bufs=8)
                            nc.sync.dma_start(
                                out=idxt[:, :cols],
                                in_=t_idxw[:, ic0[0] : ic0[0] + cols])
                            ic0[0] += cols
                            oh = sb.tile([TILE, CAP * WSC], f16,
                                         tag="oh", bufs=4)
                            tg = c["t0"] + u0
                            nc.sync.dma_start(
                                out=oh[:, : un * WSC],
                                in_=t_ohs[:, tg * WSC : (tg + un) * WSC])
                            mgT = sb.tile([F, CAP * TILE], f32,
                                          tag="mgT", bufs=4)
                            nc.gpsimd.ap_gather(
                                out_ap=mgT[:, :nidx],
                                in_ap=table[:, :],
                                idxs_ap=idxt[:, :cols],
                                channels=F, num_elems=CHUNK, d=1,
                                num_idxs=nidx)
                            dummy_gathers()
                            mgT16 = sb.tile([F, CAP * TILE], f16,
                                            tag="mgT16", bufs=4)
                            nc.vector.tensor_copy(mgT16[:, :nidx],
                                                  mgT[:, :nidx])
                            ms = sb.tile([128, CAP, F], f16,
                                         tag="ms", bufs=4)
                            nc.scalar.dma_start_transpose(
                                out=ms[:, :un, :], in_=mgT16[:, :nidx])
                            for j in range(un):
                                t = u0 + j
                                w = wins[t]
                                mm += 1
                                nc.tensor.matmul(
                                    out=sps[:F, w * WSC : (w + 1) * WSC],
                                    lhsT=ms[:, j, :],
                                    rhs=oh[:, j * WSC : (j + 1) * WSC],
                                    start=False, stop=(mm == tc_),
                                    skip_group_check=True)
                        # accumulate into y_acc (fp16)
                        nc.vector.tensor_add(
                            y_acc[:, b * BLKS : b * BLKS + bw],
                            y_acc[:, b * BLKS : b * BLKS + bw],
                            sps[:F, :bw])
                # scale by -dis (and 2x for higher orders)
                src = dism1_dram if dism_tag == 1 else dism2_dram
                for b in range(nblkc):
                    bw = blkwc[b]
                    rep = rep_row(src, b, bw)
                    nc.vector.tensor_tensor(
                        out=y_acc[:, b * BLKC : b * BLKC + bw],
                        in0=y_acc[:, b * BLKC : b * BLKC + bw],
                        in1=rep[:F, :bw],
                        op=mybir.AluOpType.mult)

            def conv_accum(w_sb, k, first, src_dram=None):
                for b in range(nblkc):
                    bw = blkwc[b]
                    if src_dram is not None:
                        rhs_t = sb.tile([F, BLKC], f16, tag="crhs", bufs=2)
                        nc.sync.dma_start(out=rhs_t[:, :bw],
                                          in_=src_dram[:, b * BLKC : b * BLKC + bw])
                        rhs = rhs_t[:, :bw]
                    else:
                        rhs = y_acc[:, b * BLKC : b * BLKC + bw]
                    cps = ps.tile([F, BLKC], f32, tag="cps", bufs=2)
                    nc.tensor.matmul(
                        out=cps[:F, :bw],
                        lhsT=w_sb[:, k * F : (k + 1) * F],
                        rhs=rhs,
                        start=True, stop=True)
                    st = sb.tile([F, BLKC], f32, tag="cst", bufs=2)
                    if first:
                        nc.vector.tensor_copy(st[:, :bw], cps[:F, :bw])
                    else:
                        nc.sync.dma_start(out=st[:, :bw],
                                          in_=oacc_dram[:, b * BLKC : b * BLKC + bw])
                        nc.vector.tensor_add(st[:, :bw], st[:, :bw], cps[:F, :bw])
                    nc.sync.dma_start(out=oacc_dram[:, b * BLKC : b * BLKC + bw],
                                      in_=st[:, :bw])

            def sub_dram(src_dram):
                for b in range(nblkc):
                    bw = blkwc[b]
                    st = sb.tile([F, BLKC], f16, tag="cst16", bufs=2)
                    nc.sync.dma_start(out=st[:, :bw],
                                      in_=src_dram[:, b * BLKC : b * BLKC + bw])
                    nc.vector.tensor_sub(y_acc[:, b * BLKC : b * BLKC + bw],
                                         y_acc[:, b * BLKC : b * BLKC + bw],
                                         st[:, :bw])

            def save_yacc(dst):
                nc.sync.dma_start(out=dst[:, :], in_=y_acc[:])

            # ---------------- layer 1 ----------------
            scale_to_u_and_ag(False, 0)               # u0 = dis*x
            conv_accum(w1_sb, 0, True, src_dram=t_xfm16)

            seg_sum_round(0, 1)                       # y_acc = Tx1
            save_yacc(tx1_dram)
            conv_accum(w1_sb, 1, False)
            scale_to_u_and_ag(True, 1)

            seg_sum_round(1, 2)                       # y_acc = 2 L Tx1
            sub_dram(t_xfm16)                         # Tx2
            conv_accum(w1_sb, 2, False)
            scale_to_u_and_ag(True, 2)

            seg_sum_round(2, 2)
            sub_dram(tx1_dram)                        # Tx3
            conv_accum(w1_sb, 3, False)

            # h1 = bn(relu(oacc + b1))
            for b in range(nblkc):
                bw = blkwc[b]
                st = sb.tile([F, BLKC], f32, tag="cst", bufs=2)
                nc.sync.dma_start(out=st[:, :bw],
                                  in_=oacc_dram[:, b * BLKC : b * BLKC + bw])
                nc.scalar.activation(
                    out=st[:, :bw], in_=st[:, :bw],
                    func=mybir.ActivationFunctionType.Relu,
                    bias=b1_sb[:, 0:1], scale=1.0)
                nc.scalar.activation(
                    out=y_acc[:, b * BLKC : b * BLKC + bw], in_=st[:, :bw],
                    func=mybir.ActivationFunctionType.Identity,
                    bias=bnbias[:, 0:1], scale=bnscale[:, 0:1])
            save_yacc(h1_dram)
            scale_to_u_and_ag(True, 3)
            conv_accum(w2_sb, 0, True)

            # ---------------- layer 2 ----------------
            seg_sum_round(3, 1)
            save_yacc(tx1_dram)
            conv_accum(w2_sb, 1, False)
            scale_to_u_and_ag(True, 4)

            seg_sum_round(4, 2)
            sub_dram(h1_dram)
            conv_accum(w2_sb, 2, False)
            scale_to_u_and_ag(True, 5)

            seg_sum_round(5, 2)
            sub_dram(tx1_dram)
            conv_accum(w2_sb, 3, False)

            ag2_in = dr.tile([F, S], f32)
            for b in range(nblkc):
                bw = blkwc[b]
                st = sb.tile([F, BLKC], f32, tag="cst", bufs=2)
                nc.sync.dma_start(out=st[:, :bw],
                                  in_=oacc_dram[:, b * BLKC : b * BLKC + bw])
                nc.scalar.activation(
                    out=st[:, :bw], in_=st[:, :bw],
                    func=mybir.ActivationFunctionType.Relu,
                    bias=b2_sb[:, 0:1], scale=1.0)
                nc.sync.dma_start(out=ag2_in[:, b * BLKC : b * BLKC + bw],
                                  in_=st[:, :bw])
            nc.gpsimd.collective_compute(
                "AllGather", mybir.AluOpType.bypass, replica_groups=rg,
                ins=[ag2_in[:]], outs=[h2_full[:, :]])

            # ---------------- pooling ----------------
            s_cols = sb.tile([F, G], f32)
            mx_cols = sb.tile([F, G], f32)
            nc.vector.memset(s_cols[:], 0.0)
            nc.vector.memset(mx_cols[:], -1e30)
            t_acc = sb.tile([F, 1], f32)
            t_m = sb.tile([F, 1], f32)
            for c in range(NC):
                # reuse the (now idle) gather table SBUF space for pooling
                nc.sync.dma_start(out=table[0:F, 0:S],
                                  in_=h2_full[c * F : (c + 1) * F, :])
                lo_n, hi_n = c * S, (c + 1) * S
                g_lo = max(int(np.searchsorted(gb, lo_n, side="right")) - 1, 0)
                for g in range(g_lo, G):
                    if int(gb[g]) >= hi_n:
                        break
                    a = max(int(gb[g]), lo_n)
                    b_ = min(int(gb[g + 1]), hi_n)
                    if a >= b_:
                        continue
                    al, bl = a - lo_n, b_ - lo_n
                    whole = int(gb[g]) >= lo_n and int(gb[g + 1]) <= hi_n
                    if whole:
                        nc.vector.tensor_reduce(
                            out=s_cols[:, g : g + 1], in_=table[0:F, al:bl],
                            axis=mybir.AxisListType.X, op=mybir.AluOpType.add)
                        nc.vector.tensor_reduce(
                            out=mx_cols[:, g : g + 1], in_=table[0:F, al:bl],
                            axis=mybir.AxisListType.X, op=mybir.AluOpType.max)
                    else:
                        nc.vector.tensor_reduce(
                            out=t_acc[:, 0:1], in_=table[0:F, al:bl],
                            axis=mybir.AxisListType.X, op=mybir.AluOpType.add)
                        nc.vector.tensor_add(s_cols[:, g : g + 1],
                                             s_cols[:, g : g + 1], t_acc[:, 0:1])
                        nc.vector.tensor_reduce(
                            out=t_m[:, 0:1], in_=table[0:F, al:bl],
                            axis=mybir.AxisListType.X, op=mybir.AluOpType.max)
                        nc.vector.tensor_tensor(
                            out=mx_cols[:, g : g + 1], in0=mx_cols[:, g : g + 1],
                            in1=t_m[:, 0:1], op=mybir.AluOpType.max)

            rc = sb.tile([1, G], f32)
            nc.vector.tensor_scalar_max(rc[:], cnt_sb[:], 1.0)
            nc.vector.reciprocal(rc[:], rc[:])
            mean_cols = sb.tile([F, G], f32)
            rep2 = ps.tile([F, G], f32, tag="rep", bufs=1)
            nc.tensor.matmul(out=rep2[:F, :G], lhsT=ones1f[:],
                             rhs=rc[0:1, :], start=True, stop=True)
            nc.vector.tensor_tensor(out=mean_cols[:], in0=s_cols[:],
                                    in1=rep2[:F, :G], op=mybir.AluOpType.mult)
            mk = sb.tile([1, G], f32)
            nc.vector.tensor_scalar(out=mk[:], in0=cnt_sb[:], scalar1=0.0,
                                    scalar2=None, op0=mybir.AluOpType.is_gt)
            rep3 = ps.tile([F, G], f32, tag="rep", bufs=1)
            nc.tensor.matmul(out=rep3[:F, :G], lhsT=ones1f[:],
                             rhs=mk[0:1, :], start=True, stop=True)
            nc.vector.tensor_tensor(out=mx_cols[:], in0=mx_cols[:],
                                    in1=rep3[:F, :G], op=mybir.AluOpType.mult)

            hps = ps.tile([2, G], f32, tag="hps")
            for ci, pc in enumerate([s_cols, mean_cols, mx_cols]):
                nc.tensor.matmul(out=hps[:2, :G],
                                 lhsT=linw_sb[:, 2 * ci : 2 * ci + 2],
                                 rhs=pc[:],
                                 start=(ci == 0), stop=(ci == 2))
            outsb = sb.tile([2, G], f32)
            nc.scalar.activation(out=outsb[:], in_=hps[:2, :G],
                                 func=mybir.ActivationFunctionType.Identity,
                                 bias=linb_sb[:, 0:1], scale=1.0)
            nc.sync.dma_start(out=t_out[:, :], in_=outsb[:])

    nc.compile()
    return nc


# ----------------------------------------------------------------------------
# Entry point
# ----------------------------------------------------------------------------

def _run(x, edge_index, edge_weight, batch, W1, b1, bn_gamma, bn_beta,
         bn_mean, bn_var, W2, b2, linW, linb, G):
    from concourse.bass_utils import run_bass_kernel_spmd

    x = np.asarray(x)
    edge_index = np.asarray(edge_index)
    ew = np.asarray(edge_weight, dtype=np.float32)
    batch = np.asarray(batch)
    N, F = x.shape
    K = int(np.asarray(W1).shape[0])
    S = N // NC
    CHUNK = N // NC

    row = edge_index[0].astype(np.int64)
    col = edge_index[1].astype(np.int64)

    eprep, cells, TOT, IDXCOLS, blkw = _prep_scatter(row, col, ew, N, S, CHUNK)
    dprep, T2, TOT2 = _prep_deg(row, ew, N, S)
    gb = np.searchsorted(batch, np.arange(G + 1))
    cnt = (gb[1:] - gb[:-1]).astype(np.float32)

    cfg = dict(N=N, S=S, F=F, G=G, K=K, CHUNK=CHUNK,
               cells=cells, TOT=TOT, IDXCOLS=IDXCOLS,
               T2=T2, TOT2=TOT2, blkw=blkw, graph_bounds=gb)
    nc = _build(cfg)

    W1a = np.asarray(W1, np.float32)
    W2a = np.asarray(W2, np.float32)
    w1in = np.ascontiguousarray(W1a.transpose(1, 0, 2).reshape(F, K * F)
                                ).astype(np.float16)
    w2in = np.ascontiguousarray(W2a.transpose(1, 0, 2).reshape(F, K * F)
                                ).astype(np.float16)
    linWa = np.asarray(linW, np.float32)
    linwt = np.concatenate([linWa[:, F * c : F * (c + 1)].T
                            for c in range(3)], axis=1)

    in_maps = []
    for i in range(NC):
        ep, dp = eprep[i], dprep[i]
        in_maps.append({
            "x_fm": np.ascontiguousarray(x[i * S : (i + 1) * S].T.astype(np.float32)),
            "x_fm16": np.ascontiguousarray(x[i * S : (i + 1) * S].T.astype(np.float16)),
            "idxw": ep["idxw"],
            "ohs": ep["ohs"],
            "ohd": dp["ohd"],
            "w1": w1in, "w2": w2in,
            "b1c": np.asarray(b1, np.float32).reshape(F, 1),
            "b2c": np.asarray(b2, np.float32).reshape(F, 1),
            "gam": np.asarray(bn_gamma, np.float32).reshape(F, 1),
            "bet": np.asarray(bn_beta, np.float32).reshape(F, 1),
            "muv": np.asarray(bn_mean, np.float32).reshape(F, 1),
            "varv": np.asarray(bn_var, np.float32).reshape(F, 1),
            "linwt": np.ascontiguousarray(linwt),
            "linbc": np.asarray(linb, np.float32).reshape(2, 1),
            "cntf": cnt.reshape(1, G),
        })

    res = run_bass_kernel_spmd(nc, in_maps, core_ids=list(range(NC)))
    out = res.results[0]["out"]
    return np.ascontiguousarray(out.T)


def kernel(x, edge_index, edge_weight, batch, W1, b1, bn_gamma, bn_beta,
           bn_mean, bn_var, W2, b2, linW, linb):
    return _run(x, edge_index, edge_weight, batch, W1, b1, bn_gamma, bn_beta,
                bn_mean, bn_var, W2, b2, linW, linb, G_FIXED)


# revision 24
# speedup vs baseline: 11.7872x; 11.7872x over previous
"""Trainium2 Bass kernel for the 2-layer ChebConv (K=4) GNN with graph pooling.

Strategy (8 NeuronCores, SPMD single program):
  - Nodes sharded into 8 contiguous slabs by destination; each core owns the
    segmented-sum rows for its slab.  Edge structure preprocessed on host into
    (dest-block x source-chunk) cells of 128-edge tiles.
  - Sparse L_hat @ h: dma_gather of source rows (int16 chunk-relative idx),
    PE matmul msgs[128,64]^T @ onehot[128,32] accumulated in PSUM [64,512]
    with per-tile dynamic window offsets (registers).
  - diag of L_hat is exactly 0 (2/lambda_max - 1), so L_hat@h is pure scatter.
  - w_e = -dis[row]*ew*dis[col]: dis folded into per-node scaling, ew into the
    one-hot values (built on device from compact uint8/f32 slot arrays).
  - deg = segment_sum(ew, row): same machinery keyed by row, ones stationary.
  - u_k = dis*Tx_k AllGathered (node-major) between rounds; everything else
    feature-major on chip.
  - Pooling: h2 AllGathered feature-major; per-graph segment reduces with
    compile-time global graph boundaries; linear head on device.
"""

import math
import numpy as np

NC = 8
BLK = 512
WSPAN = 32
TILE = 128
G_FIXED = 256


# ----------------------------------------------------------------------------
# Host-side preprocessing (integer / layout only -- no float arithmetic)
# ----------------------------------------------------------------------------

def _cut_tiles(d_rel):
    """Greedy cut of a dest-sorted array of relative dests into tiles of <=128
    edges spanning < WSPAN dests.  Returns list of (start, end) index pairs."""
    tiles = []
    n = len(d_rel)
    s = 0
    while s < n:
        e = min(s + TILE, n)
        lim = np.searchsorted(d_rel, d_rel[s] + WSPAN, side="left")
        e = min(e, int(lim))
        tiles.append((s, e))
        s = e
    return tiles


def _prep_edges(row, col, ew, N, S, chunk):
    """Per-core column-sorted structures for the segmented sums."""
    nblk = math.ceil(S / BLK)
    nch = N // chunk
    cores = []
    for i in range(NC):
        sel = np.nonzero((col // S) == i)[0]
        dloc = (col[sel] - i * S).astype(np.int64)
        order = np.argsort(dloc, kind="stable")
        sel = sel[order]
        dloc = dloc[order]
        rch = row[sel] // chunk
        blocks = []
        bbound = np.searchsorted(dloc, np.arange(nblk + 1) * BLK)
        for b in range(nblk):
            lo, hi = int(bbound[b]), int(bbound[b + 1])
            cells = []
            for ch in range(nch):
                m = np.nonzero(rch[lo:hi] == ch)[0] + lo
                d_rel = dloc[m] - b * BLK
                tiles = _cut_tiles(d_rel)
                cells.append((sel[m], d_rel, tiles))
            blocks.append(cells)
        cores.append(blocks)

    T = [[max(len(cores[i][b][ch][2]) for i in range(NC)) for ch in range(nch)]
         for b in range(nblk)]
    TOT = sum(sum(tc for tc in blk) for blk in T)
    blkw = [min(BLK, S - b * BLK) for b in range(nblk)]

    out = []
    for i in range(NC):
        gidx = np.zeros((TOT, TILE), np.int16)
        drel = np.zeros((TILE, TOT), np.uint8)
        ewv = np.zeros((TILE, TOT), np.float32)
        woff = np.zeros(TOT, np.uint32)
        t0 = 0
        for b in range(nblk):
            wclamp = max(0, blkw[b] - WSPAN)
            for ch in range(nch):
                eids_all, d_rel, tiles = cores[i][b][ch]
                for t in range(T[b][ch]):
                    tg = t0 + t
                    if t < len(tiles):
                        s, e = tiles[t]
                        k = e - s
                        eids = eids_all[s:e]
                        wo = min(int(d_rel[s]), wclamp)
                        woff[tg] = wo
                        gidx[tg, :k] = (row[eids] - ch * chunk).astype(np.int16)
                        drel[:k, tg] = (d_rel[s:e] - wo).astype(np.uint8)
                        ewv[:k, tg] = ew[eids]
                t0 += T[b][ch]
        flat = gidx.reshape(-1)  # slot order t*128+p
        idx16 = np.zeros((16, TOT * TILE // 16), np.int16)
        ssl = np.arange(TOT * TILE)
        idx16[ssl % 16, ssl // 16] = flat
        out.append(dict(idx16=idx16, drel=drel, ewv=ewv, woff=woff))
    return out, T, TOT, blkw


def _prep_deg(row, ew, N, S):
    """Per-core row-sorted structures for the degree computation."""
    nblk = math.ceil(S / BLK)
    cores = []
    for i in range(NC):
        sel = np.nonzero((row // S) == i)[0]
        rloc = (row[sel] - i * S).astype(np.int64)
        order = np.argsort(rloc, kind="stable")
        sel = sel[order]
        rloc = rloc[order]
        blocks = []
        bbound = np.searchsorted(rloc, np.arange(nblk + 1) * BLK)
        for b in range(nblk):
            lo, hi = int(bbound[b]), int(bbound[b + 1])
            d_rel = rloc[lo:hi] - b * BLK
            tiles = _cut_tiles(d_rel)
            blocks.append((sel[lo:hi], d_rel, tiles))
        cores.append(blocks)
    T2 = [max(len(cores[i][b][2]) for i in range(NC)) for b in range(nblk)]
    TOT2 = sum(T2)
    blkw = [min(BLK, S - b * BLK) for b in range(nblk)]
    out = []
    for i in range(NC):
        drel = np.zeros((TILE, TOT2), np.uint8)
        ewv = np.zeros((TILE, TOT2), np.float32)
        woff = np.zeros(TOT2, np.uint32)
        t0 = 0
        for b in range(nblk):
            wclamp = max(0, blkw[b] - WSPAN)
            m, d_rel, tiles = cores[i][b]
            for t in range(T2[b]):
                tg = t0 + t
                if t < len(tiles):
                    s, e = tiles[t]
                    k = e - s
                    wo = min(int(d_rel[s]), wclamp)
                    woff[tg] = wo
                    drel[:k, tg] = (d_rel[s:e] - wo).astype(np.uint8)
                    ewv[:k, tg] = ew[m[s:e]]
            t0 += T2[b]
        out.append(dict(drel=drel, ewv=ewv, woff=woff))
    return out, T2, TOT2, blkw


# ----------------------------------------------------------------------------
# Bass program
# ----------------------------------------------------------------------------

def _build(cfg):
    from concourse import bass, bacc, tile, mybir
    from concourse.bass import DynSlice
    from concourse.bass_types import OrderedSet
    from concourse.masks import make_identity
    import contextlib

    f32 = mybir.dt.float32
    i16 = mybir.dt.int16
    u8 = mybir.dt.uint8
    u32 = mybir.dt.uint32
    PE = mybir.EngineType.PE

    N, S, F, G = cfg["N"], cfg["S"], cfg["F"], cfg["G"]
    K = cfg["K"]
    CHUNK = cfg["CHUNK"]
    NCH = N // CHUNK
    T, TOT = cfg["T"], cfg["TOT"]
    T2, TOT2 = cfg["T2"], cfg["TOT2"]
    blkw = cfg["blkw"]
    nblk = len(blkw)
    NTL = math.ceil(S / TILE)
    gb = cfg["graph_bounds"]
    stage = cfg.get("stage", 0)
    import os
    no_coll = os.environ.get("NO_COLL", "0") == "1"
    no_gather = os.environ.get("NO_GATHER", "0") == "1"
    no_dyn = os.environ.get("NO_DYN", "0") == "1"
    gather_rounds = int(os.environ.get("GATHER_ROUNDS", "99"))
    gr_cells = int(os.environ.get("GR_CELLS", "999999"))
    gcount = [0]

    nc = bacc.Bacc("TRN2", target_bir_lowering=False, debug=False,
                   num_devices=NC)

    def din(name, shape, dt):
        return nc.dram_tensor(name, shape, dt, kind="ExternalInput")

    t_xfm = din("x_fm", [F, S], f32)
    t_idx = din("gidx", [16, TOT * TILE // 16], i16)
    t_drel = din("drel", [TILE, TOT], u8)
    t_ewv = din("ewv", [TILE, TOT], f32)
    t_woff = din("woff", [1, TOT], u32)
    t_ddrel = din("ddrel", [TILE, TOT2], u8)
    t_dewv = din("dewv", [TILE, TOT2], f32)
    t_dwoff = din("dwoff", [1, TOT2], u32)
    t_w1 = din("w1", [F, K * F], f32)
    t_w2 = din("w2", [F, K * F], f32)
    t_b1 = din("b1c", [F, 1], f32)
    t_b2 = din("b2c", [F, 1], f32)
    t_gam = din("gam", [F, 1], f32)
    t_bet = din("bet", [F, 1], f32)
    t_mu = din("muv", [F, 1], f32)
    t_var = din("varv", [F, 1], f32)
    t_linw = din("linwt", [F, 6], f32)
    t_linb = din("linbc", [2, 1], f32)
    t_cnt = din("cntf", [1, G], f32)
    t_out = nc.dram_tensor("out", [2, G], f32, kind="ExternalOutput")

    Tmax = max(max(max(tc for tc in blk) for blk in T), 16)
    t_iota = din("iotap", [TILE, Tmax * WSPAN], u8)

    rg = [list(range(NC))]

    with tile.TileContext(nc) as tc:
        ctx = contextlib.ExitStack()
        with ctx:
            sb = ctx.enter_context(tc.tile_pool(name="sb", bufs=1))
            ps = ctx.enter_context(tc.tile_pool(name="ps", bufs=1, space="PSUM"))
            dr = ctx.enter_context(tc.tile_pool(name="dr", bufs=1, space="DRAM"))

            # ---------------- persistent loads ----------------
            iota_sb = sb.tile([TILE, Tmax * WSPAN], u8)
            nc.sync.dma_start(out=iota_sb[:], in_=t_iota[:, :])
            idx_sb = sb.tile([TILE, TOT * TILE // 16], i16)
            for rpl in range(8):
                nc.sync.dma_start(out=idx_sb[16 * rpl : 16 * (rpl + 1), :],
                                  in_=t_idx[:, :])
            woff_sb = sb.tile([1, TOT], u32)
            nc.sync.dma_start(out=woff_sb[:], in_=t_woff[:, :])
            dwoff_sb = sb.tile([1, TOT2], u32)
            nc.sync.dma_start(out=dwoff_sb[:], in_=t_dwoff[:, :])
            ones_sb = sb.tile([TILE, 1], f32)
            nc.vector.memset(ones_sb[:], 1.0)
            ones1f = sb.tile([1, F], f32)
            nc.vector.memset(ones1f[:], 1.0)
            ident = sb.tile([TILE, TILE], f32)
            make_identity(nc, ident[:])
            w1_sb = sb.tile([F, K * F], f32)
            nc.sync.dma_start(out=w1_sb[:], in_=t_w1[:, :])
            w2_sb = sb.tile([F, K * F], f32)
            nc.sync.dma_start(out=w2_sb[:], in_=t_w2[:, :])
            b1_sb = sb.tile([F, 1], f32)
            nc.sync.dma_start(out=b1_sb[:], in_=t_b1[:, :])
            b2_sb = sb.tile([F, 1], f32)
            nc.sync.dma_start(out=b2_sb[:], in_=t_b2[:, :])
            linw_sb = sb.tile([F, 6], f32)
            nc.sync.dma_start(out=linw_sb[:], in_=t_linw[:, :])
            linb_sb = sb.tile([2, 1], f32)
            nc.sync.dma_start(out=linb_sb[:], in_=t_linb[:, :])
            cnt_sb = sb.tile([1, G], f32)
            nc.sync.dma_start(out=cnt_sb[:], in_=t_cnt[:, :])

            gam_sb = sb.tile([F, 1], f32)
            nc.sync.dma_start(out=gam_sb[:], in_=t_gam[:, :])
            bet_sb = sb.tile([F, 1], f32)
            nc.sync.dma_start(out=bet_sb[:], in_=t_bet[:, :])
            mu_sb = sb.tile([F, 1], f32)
            nc.sync.dma_start(out=mu_sb[:], in_=t_mu[:, :])
            var_sb = sb.tile([F, 1], f32)
            nc.sync.dma_start(out=var_sb[:], in_=t_var[:, :])
            bnscale = sb.tile([F, 1], f32)
            bnbias = sb.tile([F, 1], f32)
            tmp1 = sb.tile([F, 1], f32)
            nc.vector.tensor_scalar_add(tmp1[:], var_sb[:], 1e-5)
            nc.vector.reciprocal(tmp1[:], tmp1[:])
            nc.scalar.sqrt(tmp1[:], tmp1[:])
            nc.vector.tensor_mul(bnscale[:], gam_sb[:], tmp1[:])
            nc.vector.tensor_mul(tmp1[:], bnscale[:], mu_sb[:])
            nc.vector.tensor_sub(bnbias[:], bet_sb[:], tmp1[:])

            dis_dram = dr.tile([1, NTL * TILE], f32)
            dism1_dram = dr.tile([1, S], f32)
            dism2_dram = dr.tile([1, S], f32)
            h1_dram = dr.tile([F, S], f32)
            tx1_dram = dr.tile([F, S], f32)
            oacc_dram = dr.tile([F, S], f32)
            u_full = [dr.tile([N, F], f32, addr_space="Shared",
                              name=f"u_full{r}") for r in range(6)]
            u_loc = [dr.tile([N, F], f32, name=f"u_loc{r}") for r in range(6)]
            h2_full = dr.tile([NC * F, S], f32, addr_space="Shared",
                              name="h2_full")

            # ---------------- helpers ----------------
            def build_onehot(oh, drel_t, ew_t, Tb):
                nc.vector.tensor_tensor(
                    out=oh[:, : Tb * WSPAN],
                    in0=iota_sb[:, : Tb * WSPAN],
                    in1=drel_t.unsqueeze(-1).to_broadcast([TILE, Tb, WSPAN]),
                    op=mybir.AluOpType.is_equal)
                nc.vector.tensor_tensor(
                    out=oh[:, : Tb * WSPAN],
                    in0=oh[:, : Tb * WSPAN],
                    in1=ew_t.unsqueeze(-1).to_broadcast([TILE, Tb, WSPAN]),
                    op=mybir.AluOpType.mult)

            def load_wvals(woff_tile, t0, n):
                if no_dyn:
                    return [0] * n
                _, vals = nc.values_load_multi_w_load_instructions(
                    woff_tile[0:1, t0 : t0 + n],
                    engines=OrderedSet([PE]),
                    min_val=0, max_val=BLK - WSPAN,
                    skip_runtime_bounds_check=True)
                return vals

            # ---------------- deg phase ----------------
            t0 = 0
            for b in range(nblk):
                Tb = T2[b]
                dps = ps.tile([1, BLK], f32, tag="sps", bufs=2)
                nc.vector.memset(dps[:], 0.0)
                for gs in range(0, Tb, 16):
                    gn = min(16, Tb - gs)
                    ddrel_t = sb.tile([TILE, gn], u8, tag="drel", bufs=2)
                    nc.sync.dma_start(out=ddrel_t[:], in_=t_ddrel[:, t0 + gs : t0 + gs + gn])
                    dewv_t = sb.tile([TILE, gn], f32, tag="ewv", bufs=2)
                    nc.sync.dma_start(out=dewv_t[:], in_=t_dewv[:, t0 + gs : t0 + gs + gn])
                    doh = sb.tile([TILE, gn * WSPAN], f32, tag="oh", bufs=2)
                    build_onehot(doh, ddrel_t[:], dewv_t[:], gn)
                    vals = load_wvals(dwoff_sb, t0 + gs, gn)
                    for j in range(gn):
                        t = gs + j
                        nc.tensor.matmul(
                            out=dps[0:1, 0:WSPAN] if no_dyn else dps[0:1, DynSlice(vals[j], WSPAN)],
                            lhsT=ones_sb[:],
                            rhs=doh[:, j * WSPAN : (j + 1) * WSPAN],
                            start=False, stop=(t == Tb - 1),
                            skip_group_check=True)
                bw = blkw[b]
                drow = sb.tile([1, BLK], f32, tag="drow", bufs=2)
                mrow = sb.tile([1, BLK], f32, tag="mrow", bufs=2)
                nc.vector.tensor_scalar(
                    out=mrow[0:1, :bw], in0=dps[0:1, :bw], scalar1=0.0,
                    scalar2=None, op0=mybir.AluOpType.is_gt)
                nc.vector.tensor_scalar_max(drow[0:1, :bw], dps[0:1, :bw], 1e-30)
                nc.vector.reciprocal(drow[0:1, :bw], drow[0:1, :bw])
                nc.scalar.sqrt(drow[0:1, :bw], drow[0:1, :bw])
                nc.vector.tensor_mul(drow[0:1, :bw], drow[0:1, :bw], mrow[0:1, :bw])
                nc.sync.dma_start(out=dis_dram[0:1, b * BLK : b * BLK + bw],
                                  in_=drow[0:1, :bw])
                nc.vector.tensor_scalar_mul(mrow[0:1, :bw], drow[0:1, :bw], -1.0)
                nc.sync.dma_start(out=dism1_dram[0:1, b * BLK : b * BLK + bw],
                                  in_=mrow[0:1, :bw])
                nc.vector.tensor_scalar_mul(mrow[0:1, :bw], drow[0:1, :bw], -2.0)
                nc.sync.dma_start(out=dism2_dram[0:1, b * BLK : b * BLK + bw],
                                  in_=mrow[0:1, :bw])
                t0 += Tb

            def early_out(tag_src=None):
                outsb0 = sb.tile([2, G], f32, name="outsb0")
                nc.vector.memset(outsb0[:], 0.0)
                if tag_src is not None:
                    nc.vector.tensor_copy(outsb0[:2, : min(G, 64)],
                                          tag_src[:2, : min(G, 64)])
                nc.sync.dma_start(out=t_out[:, :], in_=outsb0[:])

            if stage == 1:
                early_out()
                nc.compile()
                return nc

            # dis node-major [128, NTL]: element (p, c) = dis[c*128 + p]
            dis_nm = sb.tile([TILE, NTL], f32)
            nc.sync.dma_start(
                out=dis_nm[:],
                in_=dis_dram[0:1, :].rearrange("o (c p) -> (o p) c", p=TILE))

            # ---------------- round helpers ----------------
            def scale_to_u_and_ag(src_fm, rnd):
                """transpose src to node-major, scale by dis per partition, AG.
                src_fm: SBUF [F, S] tile or None (stream from t_xfm)."""
                ag_in = dr.tile([S, F], f32, tag="ag_in", bufs=2, name=f"agin{rnd}")
                for c in range(NTL):
                    w = min(TILE, S - c * TILE)
                    if src_fm is None:
                        st = sb.tile([F, TILE], f32, tag="ust", bufs=3)
                        nc.sync.dma_start(out=st[:, :w],
                                          in_=t_xfm[:, c * TILE : c * TILE + w])
                        srcap = st[:, :w]
                    else:
                        srcap = src_fm[:, c * TILE : c * TILE + w]
                    tps = ps.tile([TILE, F], f32, tag="tps", bufs=2)
                    nc.tensor.transpose(out=tps[:w, :F], in_=srcap,
                                        identity=ident[:F, :F])
                    stg = sb.tile([TILE, F], f32, tag="stg", bufs=3)
                    nc.vector.tensor_scalar_mul(stg[:w, :], tps[:w, :F],
                                                dis_nm[:w, c : c + 1])
                    nc.sync.dma_start(out=ag_in[c * TILE : c * TILE + w, :],
                                      in_=stg[:w, :])
                if no_coll:
                    nc.gpsimd.dma_start(out=u_full[rnd][0 : S, :], in_=ag_in[:])
                else:
                    nc.gpsimd.collective_compute(
                        "AllGather", mybir.AluOpType.bypass, replica_groups=rg,
                        ins=[ag_in[:]], outs=[u_full[rnd][:, :]])
                nc.sync.dma_start(out=u_loc[rnd][:, :], in_=u_full[rnd][:, :])

            def seg_sum_round(rnd, dism_tag, txname):
                txt = sb.tile([F, S], f32, tag="tx", bufs=1, name=txname)
                t0 = 0
                for b in range(nblk):
                    bw = blkw[b]
                    sps = ps.tile([F, BLK], f32, tag="sps", bufs=2)
                    nc.vector.memset(sps[:], 0.0)
                    Tball = sum(T[b])
                    if Tball > 0:
                        last_ch = max(ch for ch in range(NCH) if T[b][ch] > 0)
                        tcell = t0
                        for ch in range(NCH):
                            Tc = T[b][ch]
                            if Tc == 0:
                                continue
                            drel_t = sb.tile([TILE, Tc], u8, tag="drel", bufs=2)
                            ewv_t = sb.tile([TILE, Tc], f32, tag="ewv", bufs=2)
                            nc.sync.dma_start(out=drel_t[:],
                                              in_=t_drel[:, tcell : tcell + Tc])
                            nc.sync.dma_start(out=ewv_t[:],
                                              in_=t_ewv[:, tcell : tcell + Tc])
                            oh = sb.tile([TILE, Tc * WSPAN], f32, tag="oh", bufs=2)
                            build_onehot(oh, drel_t[:], ewv_t[:], Tc)
                            ms = sb.tile([TILE, Tc, F], f32, tag="ms", bufs=2)
                            nidx = Tc * TILE
                            o16 = tcell * TILE // 16
                            gcount[0] += 1
                            if no_gather or rnd >= gather_rounds or gcount[0] > gr_cells:
                                nc.sync.dma_start(
                                    out=ms[:],
                                    in_=u_full[rnd][ch * CHUNK : ch * CHUNK + nidx, :]
                                        .rearrange("(c p) f -> p c f", p=TILE))
                            else:
                                nc.gpsimd.dma_gather(
                                    ms[:],
                                    u_loc[rnd][ch * CHUNK : (ch + 1) * CHUNK, :],
                                    idx_sb[:, o16 : o16 + nidx // 16],
                                    nidx, nidx, F,
                                    single_packet=(nidx <= 1024))
                            for ts in range(0, Tc, 16):
                                n = min(16, Tc - ts)
                                vals = load_wvals(woff_sb, tcell + ts, n)
                                for j in range(n):
                                    t = ts + j
                                    last = (ch == last_ch) and (t == Tc - 1)
                                    nc.tensor.matmul(
                                        out=sps[:F, 0:WSPAN] if no_dyn else sps[:F, DynSlice(vals[j], WSPAN)],
                                        lhsT=ms[:, t, :],
                                        rhs=oh[:, t * WSPAN : (t + 1) * WSPAN],
                                        start=False, stop=last,
                                        skip_group_check=True)
                            tcell += Tc
                    dm_t = sb.tile([1, BLK], f32, tag="dm_t", bufs=3)
                    src = dism1_dram if dism_tag == 1 else dism2_dram
                    nc.sync.dma_start(out=dm_t[0:1, :bw],
                                      in_=src[0:1, b * BLK : b * BLK + bw])
                    rep = ps.tile([F, BLK], f32, tag="rep", bufs=1)
                    nc.tensor.matmul(out=rep[:F, :bw], lhsT=ones1f[:],
                                     rhs=dm_t[0:1, :bw], start=True, stop=True)
                    rep_sb = sb.tile([F, BLK], f32, tag="rep_sb", bufs=2)
                    nc.vector.tensor_copy(rep_sb[:, :bw], rep[:F, :bw])
                    nc.vector.tensor_tensor(
                        out=txt[:, b * BLK : b * BLK + bw],
                        in0=sps[:F, :bw],
                        in1=rep_sb[:, :bw],
                        op=mybir.AluOpType.mult)
                    t0 += sum(T[b])
                return txt

            def conv_accum(tx_src, w_sb, k, first, src_dram=None):
                for b in range(nblk):
                    bw = blkw[b]
                    if src_dram is not None:
                        rhs_t = sb.tile([F, BLK], f32, tag="crhs", bufs=2)
                        nc.sync.dma_start(out=rhs_t[:, :bw],
                                          in_=src_dram[:, b * BLK : b * BLK + bw])
                        rhs = rhs_t[:, :bw]
                    else:
                        rhs = tx_src[:, b * BLK : b * BLK + bw]
                    cps = ps.tile([F, BLK], f32, tag="cps", bufs=2)
                    nc.tensor.matmul(
                        out=cps[:F, :bw],
                        lhsT=w_sb[:, k * F : (k + 1) * F],
                        rhs=rhs,
                        start=True, stop=True)
                    st = sb.tile([F, BLK], f32, tag="cst", bufs=2)
                    if first:
                        nc.vector.tensor_copy(st[:, :bw], cps[:F, :bw])
                    else:
                        nc.sync.dma_start(out=st[:, :bw],
                                          in_=oacc_dram[:, b * BLK : b * BLK + bw])
                        nc.vector.tensor_add(st[:, :bw], st[:, :bw], cps[:F, :bw])
                    nc.sync.dma_start(out=oacc_dram[:, b * BLK : b * BLK + bw],
                                      in_=st[:, :bw])

            def sub_dram(txt, src_dram):
                for b in range(nblk):
                    bw = blkw[b]
                    st = sb.tile([F, BLK], f32, tag="cst", bufs=2)
                    nc.sync.dma_start(out=st[:, :bw],
                                      in_=src_dram[:, b * BLK : b * BLK + bw])
                    nc.vector.tensor_sub(txt[:, b * BLK : b * BLK + bw],
                                         txt[:, b * BLK : b * BLK + bw],
                                         st[:, :bw])

            # ---------------- layer 1 ----------------
            scale_to_u_and_ag(None, 0)               # u0 = dis*x
            if stage == 2:
                early_out()
                nc.compile()
                return nc
            conv_accum(None, w1_sb, 0, True, src_dram=t_xfm)

            tx1 = seg_sum_round(0, 1, "tx1")
            if stage == 3:
                early_out(tx1)
                nc.compile()
                return nc
            nc.sync.dma_start(out=tx1_dram[:, :], in_=tx1[:])
            conv_accum(tx1, w1_sb, 1, False)
            scale_to_u_and_ag(tx1, 1)

            tx2 = seg_sum_round(1, 2, "tx2")
            sub_dram(tx2, t_xfm)
            conv_accum(tx2, w1_sb, 2, False)
            scale_to_u_and_ag(tx2, 2)

            tx3 = seg_sum_round(2, 2, "tx3")
            sub_dram(tx3, tx1_dram)
            conv_accum(tx3, w1_sb, 3, False)

            h1t = sb.tile([F, S], f32, tag="tx", bufs=1, name="h1t")
            for b in range(nblk):
                bw = blkw[b]
                st = sb.tile([F, BLK], f32, tag="cst", bufs=2)
                nc.sync.dma_start(out=st[:, :bw],
                                  in_=oacc_dram[:, b * BLK : b * BLK + bw])
                nc.scalar.activation(
                    out=st[:, :bw], in_=st[:, :bw],
                    func=mybir.ActivationFunctionType.Relu,
                    bias=b1_sb[:, 0:1], scale=1.0)
                nc.scalar.activation(
                    out=h1t[:, b * BLK : b * BLK + bw], in_=st[:, :bw],
                    func=mybir.ActivationFunctionType.Identity,
                    bias=bnbias[:, 0:1], scale=bnscale[:, 0:1])
            nc.sync.dma_start(out=h1_dram[:, :], in_=h1t[:])
            if stage == 4:
                early_out(h1t)
                nc.compile()
                return nc
            scale_to_u_and_ag(h1t, 3)
            conv_accum(h1t, w2_sb, 0, True)

            # ---------------- layer 2 ----------------
            tx1b = seg_sum_round(3, 1, "tx1b")
            nc.sync.dma_start(out=tx1_dram[:, :], in_=tx1b[:])
            conv_accum(tx1b, w2_sb, 1, False)
            scale_to_u_and_ag(tx1b, 4)

            tx2b = seg_sum_round(4, 2, "tx2b")
            sub_dram(tx2b, h1_dram)
            conv_accum(tx2b, w2_sb, 2, False)
            scale_to_u_and_ag(tx2b, 5)

            tx3b = seg_sum_round(5, 2, "tx3b")
            sub_dram(tx3b, tx1_dram)
            conv_accum(tx3b, w2_sb, 3, False)

            ag2_in = dr.tile([F, S], f32)
            h2t = sb.tile([F, S], f32, tag="tx", bufs=1, name="h2t")
            for b in range(nblk):
                bw = blkw[b]
                st = sb.tile([F, BLK], f32, tag="cst", bufs=2)
                nc.sync.dma_start(out=st[:, :bw],
                                  in_=oacc_dram[:, b * BLK : b * BLK + bw])
                nc.scalar.activation(
                    out=h2t[:, b * BLK : b * BLK + bw], in_=st[:, :bw],
                    func=mybir.ActivationFunctionType.Relu,
                    bias=b2_sb[:, 0:1], scale=1.0)
            nc.sync.dma_start(out=ag2_in[:, :], in_=h2t[:])
            if no_coll:
                nc.gpsimd.dma_start(out=h2_full[0:F, :], in_=ag2_in[:])
            else:
                nc.gpsimd.collective_compute(
                    "AllGather", mybir.AluOpType.bypass, replica_groups=rg,
                    ins=[ag2_in[:]], outs=[h2_full[:, :]])

            # ---------------- pooling ----------------
            s_cols = sb.tile([F, G], f32)
            mx_cols = sb.tile([F, G], f32)
            nc.vector.memset(s_cols[:], 0.0)
            nc.vector.memset(mx_cols[:], -1e30)
            t_acc = sb.tile([F, 1], f32)
            t_m = sb.tile([F, 1], f32)
            for c in range(NC):
                hch = sb.tile([F, S], f32, tag="tx", bufs=1, name=f"hch{c}")
                nc.sync.dma_start(out=hch[:], in_=h2_full[c * F : (c + 1) * F, :])
                lo_n, hi_n = c * S, (c + 1) * S
                g_lo = max(int(np.searchsorted(gb, lo_n, side="right")) - 1, 0)
                for g in range(g_lo, G):
                    if int(gb[g]) >= hi_n:
                        break
                    a = max(int(gb[g]), lo_n)
                    b_ = min(int(gb[g + 1]), hi_n)
                    if a >= b_:
                        continue
                    al, bl = a - lo_n, b_ - lo_n
                    whole = int(gb[g]) >= lo_n and int(gb[g + 1]) <= hi_n
                    if whole:
                        nc.vector.tensor_reduce(
                            out=s_cols[:, g : g + 1], in_=hch[:, al:bl],
                            axis=mybir.AxisListType.X, op=mybir.AluOpType.add)
                        nc.vector.tensor_reduce(
                            out=mx_cols[:, g : g + 1], in_=hch[:, al:bl],
                            axis=mybir.AxisListType.X, op=mybir.AluOpType.max)
                    else:
                        nc.vector.tensor_reduce(
                            out=t_acc[:, 0:1], in_=hch[:, al:bl],
                            axis=mybir.AxisListType.X, op=mybir.AluOpType.add)
                        nc.vector.tensor_add(s_cols[:, g : g + 1],
                                             s_cols[:, g : g + 1], t_acc[:, 0:1])
                        nc.vector.tensor_reduce(
                            out=t_m[:, 0:1], in_=hch[:, al:bl],
                            axis=mybir.AxisListType.X, op=mybir.AluOpType.max)
                        nc.vector.tensor_tensor(
                            out=mx_cols[:, g : g + 1], in0=mx_cols[:, g : g + 1],
                            in1=t_m[:, 0:1], op=mybir.AluOpType.max)

            rc = sb.tile([1, G], f32)
            nc.vector.tensor_scalar_max(rc[:], cnt_sb[:], 1.0)
            nc.vector.reciprocal(rc[:], rc[:])
            mean_cols = sb.tile([F, G], f32)
            rep2 = ps.tile([F, G], f32, tag="rep", bufs=1)
            nc.tensor.matmul(out=rep2[:F, :G], lhsT=ones1f[:],
                             rhs=rc[0:1, :], start=True, stop=True)
            nc.vector.tensor_tensor(out=mean_cols[:], in0=s_cols[:],
                                    in1=rep2[:F, :G], op=mybir.AluOpType.mult)
            mk = sb.tile([1, G], f32)
            nc.vector.tensor_scalar(out=mk[:], in0=cnt_sb[:], scalar1=0.0,
                                    scalar2=None, op0=mybir.AluOpType.is_gt)
            rep3 = ps.tile([F, G], f32, tag="rep", bufs=1)
            nc.tensor.matmul(out=rep3[:F, :G], lhsT=ones1f[:],
                             rhs=mk[0:1, :], start=True, stop=True)
            nc.vector.tensor_tensor(out=mx_cols[:], in0=mx_cols[:],
                                    in1=rep3[:F, :G], op=mybir.AluOpType.mult)

            hps = ps.tile([2, G], f32, tag="hps")
            for ci, pc in enumerate([s_cols, mean_cols, mx_cols]):
                nc.tensor.matmul(out=hps[:2, :G],
                                 lhsT=linw_sb[:, 2 * ci : 2 * ci + 2],
                                 rhs=pc[:],
                                 start=(ci == 0), stop=(ci == 2))
            outsb = sb.tile([2, G], f32)
            nc.scalar.activation(out=outsb[:], in_=hps[:2, :G],
                                 func=mybir.ActivationFunctionType.Identity,
                                 bias=linb_sb[:, 0:1], scale=1.0)
            nc.sync.dma_start(out=t_out[:, :], in_=outsb[:])

    nc.compile()
    return nc


# ----------------------------------------------------------------------------
# Entry point
# ----------------------------------------------------------------------------

def _run(x, edge_index, edge_weight, batch, W1, b1, bn_gamma, bn_beta,
         bn_mean, bn_var, W2, b2, linW, linb, G):
    from concourse.bass_utils import run_bass_kernel_spmd

    x = np.asarray(x)
    edge_index = np.asarray(edge_index)
    ew = np.asarray(edge_weight, dtype=np.float32)
    batch = np.asarray(batch)
    N, F = x.shape
    K = int(np.asarray(W1).shape[0])
    S = N // NC
    CHUNK = N // 4

    row = edge_index[0].astype(np.int64)
    col = edge_index[1].astype(np.int64)

    eprep, T, TOT, blkw = _prep_edges(row, col, ew, N, S, CHUNK)
    dprep, T2, TOT2, _ = _prep_deg(row, ew, N, S)
    gb = np.searchsorted(batch, np.arange(G + 1))
    cnt = (gb[1:] - gb[:-1]).astype(np.float32)

    Tmax = max(max(max(tc for tc in blk) for blk in T), 16)
    iota = np.tile((np.arange(Tmax * WSPAN) % WSPAN).astype(np.uint8),
                   (TILE, 1))

    cfg = dict(N=N, S=S, F=F, G=G, E=edge_index.shape[1], K=K, CHUNK=CHUNK,
               T=T, TOT=TOT, T2=T2, TOT2=TOT2, blkw=blkw, graph_bounds=gb)
    nc = _build(cfg)

    W1a = np.asarray(W1, np.float32)
    W2a = np.asarray(W2, np.float32)
    w1in = np.ascontiguousarray(W1a.transpose(1, 0, 2).reshape(F, K * F))
    w2in = np.ascontiguousarray(W2a.transpose(1, 0, 2).reshape(F, K * F))
    linWa = np.asarray(linW, np.float32)
    linwt = np.concatenate([linWa[:, F * c : F * (c + 1)].T
                            for c in range(3)], axis=1)

    in_maps = []
    for i in range(NC):
        ep, dp = eprep[i], dprep[i]
        in_maps.append({
            "x_fm": np.ascontiguousarray(x[i * S : (i + 1) * S].T.astype(np.float32)),
            "gidx": ep["idx16"],
            "drel": ep["drel"],
            "ewv": ep["ewv"],
            "woff": ep["woff"].reshape(1, -1),
            "ddrel": dp["drel"],
            "dewv": dp["ewv"],
            "dwoff": dp["woff"].reshape(1, -1),
            "w1": w1in, "w2": w2in,
            "b1c": np.asarray(b1, np.float32).reshape(F, 1),
            "b2c": np.asarray(b2, np.float32).reshape(F, 1),
            "gam": np.asarray(bn_gamma, np.float32).reshape(F, 1),
            "bet": np.asarray(bn_beta, np.float32).reshape(F, 1),
            "muv": np.asarray(bn_mean, np.float32).reshape(F, 1),
            "varv": np.asarray(bn_var, np.float32).reshape(F, 1),
            "linwt": np.ascontiguousarray(linwt),
            "linbc": np.asarray(linb, np.float32).reshape(2, 1),
            "cntf": cnt.reshape(1, G),
            "iotap": iota,
        })

    res = run_bass_kernel_spmd(nc, in_maps, core_ids=list(range(NC)))
    out = res.results[0]["out"]
    return np.ascontiguousarray(out.T)


def kernel(x, edge_index, edge_weight, batch, W1, b1, bn_gamma, bn_beta,
           bn_mean, bn_var, W2, b2, linW, linb):
    return _run(x, edge_index, edge_weight, batch, W1, b1, bn_gamma, bn_beta,
                bn_mean, bn_var, W2, b2, linW, linb, G_FIXED)

